# revision 4
# baseline (speedup 1.0000x reference)
"""Trainium2 Bass kernel for ClustUResNetEdgeEncoder.

Reference computation:
    cvox = data[clusts]                       # [C, V, 5]
    cnn  = concat(cvox[ei[0]], cvox[ei[1]])   # [E, 2V, 5]
    cnn[:, :, 3] = edge_id
    out  = relu(cnn.reshape(-1, 5) @ W)       # [E*2V, F]

Key identity: since column 3 is overwritten with the edge id before the
matmul, each output row is
    relu(G[vox] + eid * W[3])        with  G = data @ W0,  W0 = W w/ row3=0.
So we precompute a per-(cluster, voxel) feature table
    Gc[c, v, :] = G[clusts[c, v], :]          # [C, V, F] = [2000, 100, 16]
(6400 contiguous bytes per cluster), then each edge endpoint block is a
single 6400B gather + a fused rank-1 bias multiply-add + relu.  The kernel
is memory-bound on the endpoint gather + the 410MB output write.

Distribution across the 8 NeuronCores (SPMD):
  - Gc build is sharded: core k gathers data rows for voxel slots
    [k*25000, (k+1)*25000) of clusts.flatten(), applies W0 with DVE
    broadcast ops, and the slices are AllGathered into a full Gc copy.
  - Edges are sharded along E: core k handles edges [k*4000, (k+1)*4000)
    and writes its contiguous 1/8 of the output rows.
"""

import numpy as np

import concourse.bass as bass
import concourse.mybir as mybir
from concourse.bass import IndirectOffsetOnAxis
from concourse.bass_utils import run_bass_kernel_spmd
from concourse.tile import TileContext
from concourse.vector_clock import ScopedClock

# ---------------------------------------------------------------------------
# Problem constants (hardcoded; kernel.py must be self-contained).
N_VOX, N_CLUST, CLUST_SIZE, N_EDGE, N_FEAT = 200000, 2000, 100, 32000, 16
N_CORES = 8
E_LOC = N_EDGE // N_CORES            # 4000 edges per core
NEP = 2 * E_LOC                      # 8000 endpoint blocks per core
BLK = CLUST_SIZE * N_FEAT            # 1600 floats per endpoint block
DC = N_VOX // N_CORES                # 25000 gathered voxels per core
DC_P = 125                           # partition rows for the Gc-build tile
DC_F = DC // DC_P                    # 200 voxels per partition row
P = 128
N_TILES = (NEP + P - 1) // P         # 63 (62 full tiles + 1 tail of 64)
TAIL = NEP - (N_TILES - 1) * P       # 64

F32 = mybir.dt.float32
I32 = mybir.dt.int32


# ---------------------------------------------------------------------------
# Workaround for this neuronxcc build's per-instruction sync-wait limit:
# walrus CoreV2/V3 codegen rejects instructions carrying more than ONE sem
# wait ("Too many sync wait commands"), but Tile freely attaches several.
# Legalize after tracing: hoist extra waits onto same-engine NoOps inserted
# immediately before the instruction (same engine queue => program order).
def legalize_sync_waits(nc):
    ctr = 0
    for f in nc.m.functions:
        for bb in f.blocks:
            out = []
            for inst in bb.instructions:
                si = inst.sync_info
                if si is not None and si.on_wait and len(si.on_wait) > 1:
                    waits = list(si.on_wait)
                    si.on_wait = [waits[-1]]
                    for w in waits[:-1]:
                        ctr += 1
                        out.append(
                            mybir.InstNoOp(
                                name=f"I-waitsplit-{ctr}",
                                engine=inst.engine,
                                bass_nofuse=True,
                                sync_info=mybir.SyncInfo(on_wait=[w], on_update=[]),
                            )
                        )
                out.append(inst)
            bb.instructions = out


# ---------------------------------------------------------------------------
def build_bass():
    nc = bass.Bass(num_devices=N_CORES)

    data_ext = nc.dram_tensor("data", [N_VOX, 5], F32, kind="ExternalInput")
    dcidx_ext = nc.dram_tensor("dc_idx", [DC_P, DC_F], I32, kind="ExternalInput")
    epidx_ext = nc.dram_tensor("ep_idx", [P, N_TILES], I32, kind="ExternalInput")
    eids_ext = nc.dram_tensor("eids", [P, N_TILES], F32, kind="ExternalInput")
    w0_ext = nc.dram_tensor("w0rep", [P, 80], F32, kind="ExternalInput")
    w3_ext = nc.dram_tensor("w3rep", [P, 16], F32, kind="ExternalInput")
    out_ext = nc.dram_tensor("out", [NEP, BLK], F32, kind="ExternalOutput")

    gc_slice = nc.dram_tensor("gc_slice", [DC_P, DC_F * N_FEAT], F32)
    gc_full = nc.dram_tensor("gc_full", [N_CLUST, BLK], F32, addr_space="Shared")

    mult = mybir.AluOpType.mult
    add = mybir.AluOpType.add

    with TileContext(nc) as tc:
        with (
            tc.tile_pool(name="const", bufs=1) as cpool,
            tc.tile_pool(name="build", bufs=1) as bpool,
            tc.tile_pool(name="g", bufs=3) as gpool,
            tc.tile_pool(name="s", bufs=3) as spool,
            tc.tile_pool(name="o", bufs=3) as opool,
        ):
            # ---- constants -------------------------------------------------
            dcidx = cpool.tile([DC_P, DC_F], I32)
            nc.sync.dma_start(out=dcidx[:], in_=dcidx_ext[:])
            epidx = cpool.tile([P, N_TILES], I32)
            nc.sync.dma_start(out=epidx[:], in_=epidx_ext[:])
            eids = cpool.tile([P, N_TILES], F32)
            nc.sync.dma_start(out=eids[:], in_=eids_ext[:])
            w0 = cpool.tile([P, 80], F32)
            nc.sync.dma_start(out=w0[:], in_=w0_ext[:])
            w3 = cpool.tile([P, 16], F32)
            nc.sync.dma_start(out=w3[:], in_=w3_ext[:])

            # ---- Gc build (sharded): gather data rows, apply W0 ------------
            dc = bpool.tile([DC_P, DC_F * 5], F32)
            nc.gpsimd.indirect_dma_start(
                out=dc[:],
                out_offset=None,
                in_=data_ext[:],
                in_offset=IndirectOffsetOnAxis(ap=dcidx[:], axis=0),
            )
            gc = bpool.tile([DC_P, DC_F * N_FEAT], F32)
            tmp = bpool.tile([DC_P, DC_F * N_FEAT], F32)
            d3 = dc[:].rearrange("p (v k) -> p v k", k=5)
            w03 = w0[:DC_P, :].rearrange("p (k n) -> p k n", n=16)
            gc3 = gc[:].rearrange("p (v n) -> p v n", n=16)
            tmp3 = tmp[:].rearrange("p (v n) -> p v n", n=16)
            for k in range(5):
                a = d3[:, :, k : k + 1].to_broadcast([DC_P, DC_F, 16])
                b = w03[:, k : k + 1, :].to_broadcast([DC_P, DC_F, 16])
                if k == 0:
                    nc.vector.tensor_tensor(out=gc3, in0=a, in1=b, op=mult)
                else:
                    nc.vector.tensor_tensor(out=tmp3, in0=a, in1=b, op=mult)
                    nc.vector.tensor_add(out=gc3, in0=gc3, in1=tmp3)
            nc.sync.dma_start(out=gc_slice[:], in_=gc[:])
            nc.gpsimd.collective_compute(
                "AllGather",
                mybir.AluOpType.bypass,
                replica_groups=[list(range(N_CORES))],
                ins=[gc_slice[:]],
                outs=[gc_full[:]],
            )

            # ---- main loop: per 128 endpoint blocks ------------------------
            for t in range(N_TILES):
                p = P if t < N_TILES - 1 else TAIL
                g = gpool.tile([P, BLK], F32)
                nc.gpsimd.indirect_dma_start(
                    out=g[:p, :],
                    out_offset=None,
                    in_=gc_full[:],
                    in_offset=IndirectOffsetOnAxis(ap=epidx[:p, t : t + 1], axis=0),
                )
                s = spool.tile([P, BLK], F32)
                g3 = g[:p, :].rearrange("p (v n) -> p v n", n=16)
                s3 = s[:p, :].rearrange("p (v n) -> p v n", n=16)
                w3b = (
                    w3[:p, :]
                    .rearrange("p (v n) -> p v n", v=1)
                    .to_broadcast([p, CLUST_SIZE, 16])
                )
                # s = (w3 * eid_p) + g   — fused rank-1 bias add
                nc.vector.scalar_tensor_tensor(
                    out=s3,
                    in0=w3b,
                    scalar=eids[:p, t : t + 1],
                    in1=g3,
                    op0=mult,
                    op1=add,
                )
                o = opool.tile([P, BLK], F32)
                nc.vector.tensor_relu(out=o[:p, :], in_=s[:p, :])
                nc.sync.dma_start(
                    out=out_ext[t * P : t * P + p, :], in_=o[:p, :]
                )

    legalize_sync_waits(nc)
    return nc


# ---------------------------------------------------------------------------
def make_in_maps(data, clusts, edge_index, W):
    data = np.ascontiguousarray(np.asarray(data, dtype=np.float32))
    clusts_flat = np.asarray(clusts).reshape(-1).astype(np.int32)
    ei = np.asarray(edge_index).astype(np.int32)
    W = np.asarray(W, dtype=np.float32)

    W0 = W.copy()
    W0[3, :] = 0.0
    w0rep = np.ascontiguousarray(
        np.broadcast_to(W0.reshape(1, 80), (P, 80)), dtype=np.float32
    )
    w3rep = np.ascontiguousarray(
        np.broadcast_to(W[3].reshape(1, 16), (P, 16)), dtype=np.float32
    )

    in_maps = []
    for k in range(N_CORES):
        dc_idx = clusts_flat[k * DC : (k + 1) * DC].reshape(DC_P, DC_F)
        e0 = k * E_LOC
        ep = np.empty(NEP, dtype=np.int32)
        ep[0::2] = ei[0, e0 : e0 + E_LOC]
        ep[1::2] = ei[1, e0 : e0 + E_LOC]
        ee = np.repeat(np.arange(e0, e0 + E_LOC, dtype=np.float32), 2)
        ep_pad = np.zeros(N_TILES * P, dtype=np.int32)
        ep_pad[:NEP] = ep
        ee_pad = np.zeros(N_TILES * P, dtype=np.float32)
        ee_pad[:NEP] = ee
        in_maps.append(
            {
                "data": data,
                "dc_idx": np.ascontiguousarray(dc_idx),
                "ep_idx": np.ascontiguousarray(ep_pad.reshape(N_TILES, P).T),
                "eids": np.ascontiguousarray(ee_pad.reshape(N_TILES, P).T),
                "w0rep": w0rep,
                "w3rep": w3rep,
            }
        )
    return in_maps


_NC_CACHE = None


def kernel(data, clusts, edge_index, W):
    global _NC_CACHE
    if _NC_CACHE is None:
        _NC_CACHE = build_bass()
    nc = _NC_CACHE
    in_maps = make_in_maps(data, clusts, edge_index, W)
    res = run_bass_kernel_spmd(nc, in_maps, list(range(N_CORES)))
    out = np.concatenate(
        [res.results[c]["out"].reshape(-1, N_FEAT) for c in range(N_CORES)], axis=0
    )
    return out
